# revision 18
# baseline (speedup 1.0000x reference)
"""Trainium2 Bass kernel for top-2 MoE routing (nn_JaxMoE_26431228740246).

Strategy: expert parallel across 8 NeuronCores with SPARSE dispatch and
overflow load-balancing.  The reference computes every expert densely over
all 2048 tokens, but only the top-2 experts per token carry nonzero combine
weight, so each core only needs its assigned tokens (~512 avg per expert).

Each core runs two token streams with independent weight sets (delivered
per-core via in_maps, so one SPMD program serves all cores):
  stream 0 (primary):  the core's own expert, first <=512 tokens
  stream 1 (overflow): spill tokens (load-512) of overloaded experts,
                       packed into a small fixed chunk (c2, e.g. 20 wide)
This caps PE moving rows at 512+c2 per f-tile instead of max-load (551).

Host side: router softmax + top-2 + renormalize in fp32 numpy, gather into
per-partition-contiguous slabs, assign overflow slices to cores, scatter-add
the weighted expert outputs back to [T, D].

Device side (per core): SwiGLU MLP in bf16 — h = silu(x@Wg)*(x@Wu);
out = h @ Wd — weights streamed over both HWDGE queues (SP + Activation),
fp32 PSUM accumulation.  The first gate/up block is split small (128
f-cols) so the PE starts after ~1MB of DMA, and a short chain of dummy
matmuls on zeroed scratch warms the TRN2 PE p-state ramp while the first
DMAs land.  No router, no transpose, no collectives on device.

Shapes (hardcoded): T=2048, D=1024, F=4096, E=8, K=2.
"""

import os
import sys

import numpy as np
import ml_dtypes


def _ensure_path():
    for p in (
        "/root/.axon_site",
        "/root/.axon_site/_ro/trn_rl_repo",
        "/root/.axon_site/_ro/pypackages",
        "/opt/trn_rl_repo",
    ):
        if os.path.isdir(p) and p not in sys.path:
            sys.path.append(p)


_ensure_path()

T, D, F, E = 2048, 1024, 4096, 8
DT = D // 128       # 8 d-tiles
FTILES = F // 128   # 32 f-tiles
DC = 128            # d columns per down-proj weight DMA chunk
NDC = D // DC

# gate/up weight stream blocks (f-offset, width): first block small so the
# first MM group starts early
W_BLOCKS = [(0, 128), (128, 384)] + [(f0, 512) for f0 in range(512, F, 512)]

_CACHE = {}


def _xpieces(c1):
    out, t0 = [], 0
    while t0 < c1:
        tw = min(128, c1 - t0)
        out.append((t0, tw))
        t0 += tw
    return out


def _build(widths):
    """widths: tuple of per-stream token-chunk widths, e.g. (512, 20)."""
    import concourse.tile as tile
    from concourse import bacc, mybir

    fp32 = mybir.dt.float32
    bf16 = mybir.dt.bfloat16
    Act = mybir.ActivationFunctionType

    NS = len(widths)
    nc = bacc.Bacc("TRN2", target_bir_lowering=False, debug=False, num_devices=E)

    # primary x slab arrives as small 128-token pieces so the very first MM
    # sub-group only waits on ~0.26MB of DMA
    xpieces = _xpieces(widths[0])
    xps = [
        nc.dram_tensor(f"xp{i}", [128, DT, tw], bf16, kind="ExternalInput").ap()
        for i, (t0, tw) in enumerate(xpieces)
    ]

    # every input tensor is one contiguous DMA block (host pre-permuted)
    xls, wgs, wus, wdls, outs = [], [], [], [], []
    for s, w in enumerate(widths):
        xls.append(
            nc.dram_tensor(f"xl{s}", [128, DT, w], bf16, kind="ExternalInput").ap()
            if s > 0
            else None
        )
        wgs.append(
            [
                nc.dram_tensor(
                    f"wg{s}_{j}", [128, DT, bw], bf16, kind="ExternalInput"
                ).ap()
                for j, (f0, bw) in enumerate(W_BLOCKS)
            ]
        )
        wus.append(
            [
                nc.dram_tensor(
                    f"wu{s}_{j}", [128, DT, bw], bf16, kind="ExternalInput"
                ).ap()
                for j, (f0, bw) in enumerate(W_BLOCKS)
            ]
        )
        wdls.append(
            nc.dram_tensor(
                f"wd{s}", [128, NDC, FTILES, DC], bf16, kind="ExternalInput"
            ).ap()
        )
        outs.append(
            nc.dram_tensor(f"out{s}", [128, DT, w], fp32, kind="ExternalOutput").ap()
        )

    from contextlib import ExitStack

    with tile.TileContext(nc) as tc, ExitStack() as ctx:
        pconst = ctx.enter_context(tc.tile_pool(name="const", bufs=1))
        ph = ctx.enter_context(tc.tile_pool(name="h", bufs=1))
        pwsm = ctx.enter_context(tc.tile_pool(name="wsm", bufs=1))
        pw512 = ctx.enter_context(tc.tile_pool(name="w512", bufs=2))
        pwd = ctx.enter_context(tc.tile_pool(name="wd", bufs=3))
        posb = ctx.enter_context(tc.tile_pool(name="osb", bufs=2))
        ptmp = ctx.enter_context(tc.tile_pool(name="tmp", bufs=2))
        pwarm = ctx.enter_context(tc.tile_pool(name="warm", bufs=1, space="PSUM"))
        pmm = {
            w: ctx.enter_context(
                tc.tile_pool(name=f"mm{w}", bufs=4 if w >= 512 else 3, space="PSUM")
            )
            for w in sorted(set(widths))
        }

        # x slab tile; piece 0 (tokens 0:128) on Act while the first gate
        # weight piece goes on SP — the first MM sub-group needs only those
        xsb = [None] * NS
        xsb[0] = pconst.tile([128, DT, widths[0]], bf16, tag="xsb0", name="xsb0")

        wg0_t = pwsm.tile([128, DT, W_BLOCKS[0][1]], bf16, tag="wg128_0")
        nc.sync.dma_start(wg0_t[:], wgs[0][0][:])
        t0_, tw_ = xpieces[0]
        nc.scalar.dma_start(xsb[0][:, :, t0_ : t0_ + tw_], xps[0][:])
        wu0_t = pwsm.tile([128, DT, W_BLOCKS[0][1]], bf16, tag="wu128_0")
        nc.scalar.dma_start(wu0_t[:], wus[0][0][:])
        # remaining x pieces alternate queues right behind the first deps
        for i, (t0, tw) in enumerate(xpieces[1:], start=1):
            eng = nc.sync if i % 2 == 1 else nc.scalar
            eng.dma_start(xsb[0][:, :, t0 : t0 + tw], xps[i][:])

        # PE warm-up on zeroed scratch while the first DMAs land (TRN2 PE
        # p-state ramp needs ~3us of sustained activity to hit 2.4 GHz)
        wz = pconst.tile([128, 128], bf16, tag="wz")
        nc.vector.memzero(wz[:])
        mz = pconst.tile([128, 256], bf16, tag="mz")
        nc.vector.memzero(mz[:])
        pwv = pwarm.tile([128, 256], fp32, tag="warm")
        NWARM = 8
        for i in range(NWARM):
            nc.tensor.matmul(
                pwv[:], wz[:], mz[:], start=(i == 0), stop=(i == NWARM - 1)
            )

        hs = [
            ph.tile([128, FTILES, w], bf16, tag=f"h{s}", name=f"h{s}")
            for s, w in enumerate(widths)
        ]

        def gate_up(s, k, wg_ap, wu_ap, t0, cw, tag):
            pg = pmm[tag].tile([128, cw], fp32, tag=f"mm{tag}", name="pg")
            for do in range(DT):
                nc.tensor.matmul(
                    pg[:],
                    wg_ap[:, do, :],
                    xsb[s][:, do, t0 : t0 + cw],
                    start=(do == 0),
                    stop=(do == DT - 1),
                )
            tmp = ptmp.tile([128, cw], fp32, tag=f"tmp{tag}", name="tmp")
            nc.scalar.activation(tmp[:], pg[:], Act.Silu)
            pu = pmm[tag].tile([128, cw], fp32, tag=f"mm{tag}", name="pu")
            for do in range(DT):
                nc.tensor.matmul(
                    pu[:],
                    wu_ap[:, do, :],
                    xsb[s][:, do, t0 : t0 + cw],
                    start=(do == 0),
                    stop=(do == DT - 1),
                )
            nc.vector.tensor_mul(hs[s][:, k, t0 : t0 + cw], tmp[:], pu[:])

        # ---- gate/up -> h ----
        for j, (f0, bw) in enumerate(W_BLOCKS):
            wg_t, wu_t = [], []
            for s in range(NS):
                if j == 0 and s == 0:
                    wg_t.append(wg0_t)
                    wu_t.append(wu0_t)
                    continue
                pool = pw512 if bw >= 512 else pwsm
                g_t = pool.tile([128, DT, bw], bf16, tag=f"wg{bw}_{s}")
                nc.sync.dma_start(g_t[:], wgs[s][j][:])
                u_t = pool.tile([128, DT, bw], bf16, tag=f"wu{bw}_{s}")
                nc.scalar.dma_start(u_t[:], wus[s][j][:])
                wg_t.append(g_t)
                wu_t.append(u_t)
                if j == 0 and s == 1:
                    # overflow x slab: needed from the second MM group on
                    xsb[1] = pconst.tile(
                        [128, DT, widths[1]], bf16, tag="xsb1", name="xsb1"
                    )
                    nc.sync.dma_start(
                        xsb[1][:, : DT // 2, :], xls[1][:, : DT // 2, :]
                    )
                    nc.scalar.dma_start(
                        xsb[1][:, DT // 2 :, :], xls[1][:, DT // 2 :, :]
                    )
            for fi in range(bw // 128):
                k = (f0 // 128) + fi
                for s, cw in enumerate(widths):
                    wslice = (
                        wg_t[s][:, :, fi * 128 : (fi + 1) * 128],
                        wu_t[s][:, :, fi * 128 : (fi + 1) * 128],
                    )
                    if k == 0 and s == 0:
                        # first f-tile: 128-token sub-groups so the PE can
                        # start while the rest of the x slab streams in
                        for t0, tw in xpieces:
                            gate_up(s, k, *wslice, t0, tw, cw)
                    else:
                        gate_up(s, k, *wslice, 0, cw, cw)

        # ---- down-projection ----
        for dp in range(NDC):
            wd_t = []
            for s in range(NS):
                t = pwd.tile([128, FTILES, DC], bf16, tag=f"wd{s}")
                dma_eng = nc.sync if (dp + s) % 2 == 0 else nc.scalar
                dma_eng.dma_start(t[:], wdls[s][:, dp])
                wd_t.append(t)
            for di in range(DC // 128):
                dd = dp * (DC // 128) + di
                last_dd = dd == D // 128 - 1
                order = (
                    list(reversed(range(NS))) if last_dd else list(range(NS))
                )
                for s in order:
                    cw = widths[s]
                    # the very last (wide) output: pipeline in 128-col
                    # pieces so the final DMA is small and starts early
                    pieces = (
                        [(o, 128) for o in range(0, cw, 128)]
                        if last_dd and cw >= 512
                        else [(0, cw)]
                    )
                    for o, pw in pieces:
                        po = pmm[cw].tile([128, pw], fp32, tag=f"mm{cw}")
                        for k in range(FTILES):
                            nc.tensor.matmul(
                                po[:],
                                wd_t[s][:, k, di * 128 : (di + 1) * 128],
                                hs[s][:, k, o : o + pw],
                                start=(k == 0),
                                stop=(k == FTILES - 1),
                            )
                        osb = posb.tile([128, pw], fp32, tag=f"osb{cw}")
                        nc.vector.tensor_copy(osb[:], po[:])
                        nc.sync.dma_start(outs[s][:, dd, o : o + pw], osb[:])

    nc.compile()
    return nc


def _get_nc(widths):
    key = ("nc", widths)
    if key not in _CACHE:
        _CACHE[key] = _build(widths)
    return _CACHE[key]


_BF = ml_dtypes.bfloat16


def _wblock(w_DF, f0, w):
    # [D, f0:f0+w] -> [128, DT, w] partition layout, contiguous
    return np.ascontiguousarray(
        w_DF[:, f0 : f0 + w].reshape(DT, 128, w).transpose(1, 0, 2)
    ).astype(_BF)


def _xslab(xT_DL, cw):
    # [D, L<=cw] -> zero-padded [128, DT, cw] partition layout
    xe = np.zeros((128, DT, cw), dtype=_BF)
    n = xT_DL.shape[1]
    if n:
        xe[:, :, :n] = xT_DL.reshape(DT, 128, n).transpose(1, 0, 2)
    return xe


def kernel(
    x_TD, w_router_DE, kernel_gating_EDF, kernel_up_proj_EDF, kernel_down_proj_EFD
):
    from concourse.bass_utils import run_bass_kernel_spmd

    x = np.ascontiguousarray(np.asarray(x_TD, dtype=np.float32))
    wr = np.ascontiguousarray(np.asarray(w_router_DE, dtype=np.float32))
    g = np.asarray(kernel_gating_EDF, dtype=np.float32)
    u = np.asarray(kernel_up_proj_EDF, dtype=np.float32)
    d = np.asarray(kernel_down_proj_EFD, dtype=np.float32)

    # ---- router (fp32, exact top-2 + renormalize) ----
    logits = x @ wr
    p = np.exp(logits - logits.max(axis=-1, keepdims=True))
    p /= p.sum(axis=-1, keepdims=True)
    rows = np.arange(T)
    i1 = p.argmax(axis=-1)
    p2 = p.copy()
    p2[rows, i1] = -1.0
    i2 = p2.argmax(axis=-1)
    v1, v2 = p[rows, i1], p[rows, i2]
    s = v1 + v2
    w1, w2 = v1 / s, v2 / s

    idxs, wts = [], []
    for e in range(E):
        m1 = i1 == e
        sel = m1 | (i2 == e)
        idx = np.nonzero(sel)[0]
        idxs.append(idx)
        wts.append(np.where(m1, w1, w2)[idx].astype(np.float32))

    loads = [len(ix) for ix in idxs]
    L = max(loads)

    # ---- choose stream widths: primary c1 (<=512) + overflow c2 ----
    if L <= 512:
        widths = (max(128, -(-L // 8) * 8),)
        over_asn = []
    else:
        c1 = 512
        c2 = None
        for cand in range(8, 513, 4):
            slots = sum(-(-max(0, l - c1) // cand) for l in loads)
            if slots <= E:
                c2 = cand
                break
        assert c2 is not None
        widths = (c1, c2)
        # assign overflow slices (expert, tok_start, tok_end) to cores
        over_asn = [None] * E  # per core: (expert, start, end) into idxs[e]
        slot = 0
        for e in sorted(range(E), key=lambda e: loads[e], reverse=True):
            o = loads[e] - c1
            start = c1
            while o > 0:
                take = min(o, c2)
                over_asn[slot] = (e, start, start + take)
                slot += 1
                start += take
                o -= take

    nc = _get_nc(widths)

    zero_w = None
    xpieces = _xpieces(widths[0])
    in_maps = []
    for core in range(E):
        m = {}
        # stream 0: this core's own expert, first c1 tokens, in 128-token
        # pieces matching the device's sub-chunked first f-tile
        e0 = core
        n0 = min(loads[e0], widths[0])
        slab0 = _xslab(x[idxs[e0][:n0]].T, widths[0])
        for i, (t0, tw) in enumerate(xpieces):
            m[f"xp{i}"] = np.ascontiguousarray(slab0[:, :, t0 : t0 + tw])
        for j, (f0, bw) in enumerate(W_BLOCKS):
            m[f"wg0_{j}"] = _wblock(g[e0], f0, bw)
            m[f"wu0_{j}"] = _wblock(u[e0], f0, bw)
        m["wd0"] = (
            d[e0].reshape(FTILES, 128, NDC, DC).transpose(1, 2, 0, 3).astype(_BF)
        )
        # stream 1: overflow slice of some (possibly other) expert
        if len(widths) > 1:
            asn = over_asn[core]
            if asn is not None:
                e1, t0, t1 = asn
                m["xl1"] = _xslab(x[idxs[e1][t0:t1]].T, widths[1])
                for j, (f0, bw) in enumerate(W_BLOCKS):
                    m[f"wg1_{j}"] = _wblock(g[e1], f0, bw)
                    m[f"wu1_{j}"] = _wblock(u[e1], f0, bw)
                m["wd1"] = (
                    d[e1]
                    .reshape(FTILES, 128, NDC, DC)
                    .transpose(1, 2, 0, 3)
                    .astype(_BF)
                )
            else:
                if zero_w is None:
                    zero_w = {
                        "xl1": np.zeros((128, DT, widths[1]), dtype=_BF),
                        **{
                            f"wg1_{j}": np.zeros((128, DT, bw), dtype=_BF)
                            for j, (f0, bw) in enumerate(W_BLOCKS)
                        },
                        **{
                            f"wu1_{j}": np.zeros((128, DT, bw), dtype=_BF)
                            for j, (f0, bw) in enumerate(W_BLOCKS)
                        },
                        "wd1": np.zeros((128, NDC, FTILES, DC), dtype=_BF),
                    }
                m.update(zero_w)
        in_maps.append(m)

    trace = bool(os.environ.get("BASS_PROF"))
    try:
        res = run_bass_kernel_spmd(nc, in_maps, list(range(E)), trace=trace)
    except Exception:
        if not trace:
            raise
        res = run_bass_kernel_spmd(nc, in_maps, list(range(E)), trace=False)
    _CACHE["last_result"] = res

    out = np.zeros((T, D), dtype=np.float32)
    for core in range(E):
        e0 = core
        n0 = min(loads[e0], widths[0])
        y0 = np.asarray(res.results[core]["out0"], dtype=np.float32)
        y0 = y0.transpose(1, 0, 2).reshape(D, widths[0])
        out[idxs[e0][:n0]] += wts[e0][:n0, None] * y0[:, :n0].T
        if len(widths) > 1 and over_asn[core] is not None:
            e1, t0, t1 = over_asn[core]
            y1 = np.asarray(res.results[core]["out1"], dtype=np.float32)
            y1 = y1.transpose(1, 0, 2).reshape(D, widths[1])
            n1 = t1 - t0
            out[idxs[e1][t0:t1]] += wts[e1][t0:t1, None] * y1[:, :n1].T
    return out
